# revision 15
# baseline (speedup 1.0000x reference)
"""Trainium2 Bass kernel for NeuralDisCoCirc forward pass.

Problem: L=8 sequential layers; each layer, per sample b:
    z = h @ W[l,b] + bias[l,b];  h = where(mask[l,b], relu(z), z)
Shapes: x [16,1024] f32, weights [8,16,1024,1024] f32,
        biases/masks [8,16,1024].

Strategy (data-parallel over batch, 2 samples per core, 8 cores):
  - Weights stream as bf16 (host cast; rel err ~4e-3 over 8 layers,
    well under the 2e-2 gate) on the SINGLE sync HWDGE ring: per-layer
    tiles are [128 x 8KB] contiguous rows, 8-deep prefetch so the
    stream never stalls on pool-slot release.  The scalar HWDGE ring
    carries ONLY the small row->column scatters + final outputs, so
    they round-robin against the weight queue at packet granularity
    (~0.5us) instead of queueing behind it (~3.5us on SWDGE).
  - h is kept column-major ([p, c], element i = p*KI + c) as the
    matmul stationary operand.  Per layer: 32 accumulating matmuls
    (h chunk [128,1] stationary, W chunk [128,256] moving) close z in
    QUARTER ranges, so each quarter's chain (bias-fused PSUM->SBUF row
    copy on DVE -> [32,8] scatter on scalar ring -> masked-relu on the
    column quarter) pipelines under the remaining matmuls.  Only the
    last quarter's short chain (~1.5-2.5us) separates layer l from
    layer l+2 (same sample), under the ~3.4us the other sample's
    matmul group provides - so the PE stays fed and HAM stays warm.
  - bias is added in ROW space during the PSUM->SBUF copy (the copy
    becomes a tensor_add), so the column side needs only 2 DVE ops:
    h = zb - mask*min(zb, 0).
  - The final layer skips the column layout: bias + masked relu run in
    row layout per jb half and the output ships row-contiguous on the
    scalar ring.
  - Memory-bound: 32 MB of bf16 weights per core at ~358 GB/s/core.
"""

import numpy as np

import concourse.bass as bass
import concourse.mybir as mybir
from concourse import bacc
from concourse.tile import TileContext
from concourse.bass_utils import run_bass_kernel_spmd

L = 8          # layers
B = 16         # full batch
D = 1024       # width
NCORES = 8
BC = B // NCORES   # samples per core (2)
NT = L * BC        # (layer, sample) tiles per core (16)
KI = D // 128      # 8 chunks of 128 along the contraction dim
P = 128
NQ = 4             # z closes in quarters of 256
QW = D // NQ       # 256

F32 = mybir.dt.float32
F32R = mybir.dt.float32r
BF16 = mybir.dt.bfloat16

WMODE = "bf16"

_CACHE = {}


def _build(wmode: str) -> bass.Bass:
    wdt = {"bf16": BF16, "f32r": F32R, "f32": F32}[wmode]
    hdt = {"bf16": BF16, "f32r": F32R, "f32": F32}[wmode]

    nc = bacc.Bacc("TRN2", target_bir_lowering=False, debug=False)
    w = nc.declare_dram_parameter("w", [NT, P, KI * D], wdt, isOutput=False)
    x = nc.declare_dram_parameter("x", [P, BC * KI], hdt, isOutput=False)
    # bias rows for all tiles (bf16 to halve SBUF footprint); masks col layout
    br = nc.declare_dram_parameter("br", [1, NT * D], BF16, isOutput=False)
    mk = nc.declare_dram_parameter("mk", [P, NT * KI], F32, isOutput=False)
    # last layer's bias/mask in ROW layout: bmr[b] = [bias_row | mask_row]
    bmr = nc.declare_dram_parameter("bmr", [BC, 2 * D], F32, isOutput=False)
    out = nc.declare_dram_parameter("out", [BC, D], F32, isOutput=True)

    with TileContext(nc) as tc:
        with (
            tc.tile_pool(name="wp", bufs=8) as wp,
            tc.tile_pool(name="const", bufs=1) as cp,
            tc.tile_pool(name="hrow", bufs=2) as hrp,
            tc.tile_pool(name="hcol", bufs=4) as hcp,
            tc.tile_pool(name="psr", bufs=4, space="PSUM") as psr,
        ):
            # Small SWDGE input loads first: they ride their own engine
            # (gpsimd) and land while the HWDGE weight flood is still in
            # descriptor generation.
            brt = cp.tile([1, NT * D], BF16, tag="br")
            mkt = cp.tile([P, NT * KI], F32, tag="mk")
            bmrt = cp.tile([1, BC * 2 * D], F32, tag="bmr")
            xt = cp.tile([P, BC * KI], hdt, tag="x")
            nc.gpsimd.dma_start(out=xt, in_=x[:])
            nc.gpsimd.dma_start(out=brt, in_=br[:])
            nc.gpsimd.dma_start(out=mkt, in_=mk[:])
            nc.gpsimd.dma_start(
                out=bmrt, in_=bmr[:].rearrange("b n -> () (b n)"))

            KH = KI // 2  # ki chunks per half-tile
            LAST = NT - 1
            wtiles = {}
            for t in range(NT):
                if t == 0:
                    # small starter blocks so the first weight bytes land
                    # earlier (fewer descriptors before the ring fires)
                    ST = D
                    wa = wp.tile([P, KH * D], wdt, tag="wa")
                    wb = wp.tile([P, KH * D], wdt, tag="wb")
                    nc.sync.dma_start(out=wa[:, :ST], in_=w[t, :, :ST])
                    nc.sync.dma_start(
                        out=wb[:, :ST], in_=w[t, :, KH * D:KH * D + ST])
                    nc.sync.dma_start(out=wa[:, ST:], in_=w[t, :, ST:KH * D])
                    nc.sync.dma_start(
                        out=wb[:, ST:], in_=w[t, :, KH * D + ST:])
                    wtiles[t] = (wa, wb)
                elif t < LAST:
                    wa = wp.tile([P, KH * D], wdt, tag="wa")
                    wb = wp.tile([P, KH * D], wdt, tag="wb")
                    nc.sync.dma_start(out=wa, in_=w[t, :, : KH * D])
                    nc.sync.dma_start(out=wb, in_=w[t, :, KH * D:])
                    wtiles[t] = (wa, wb)
                else:
                    # last tile: host re-laid it jb-major
                    # ([p, jb*4096 + ki*512 + j']), streamed as 8
                    # contiguous blocks so the jb0 PSUM group closes at
                    # the tile's halfway point.
                    qs = []
                    for q in range(8):
                        wq = wp.tile([P, KH * D], wdt,
                                     tag=("wa" if q % 2 == 0 else "wb"))
                        nc.sync.dma_start(
                            out=wq[:, :D],
                            in_=w[t, :, q * D:(q + 1) * D],
                        )
                        qs.append(wq[:, :D])
                    wtiles[t] = tuple(qs)

            h = [xt[:, b * KI:(b + 1) * KI] for b in range(BC)]

            for l in range(L):
                for b in range(BC):
                    t = l * BC + b
                    cur = h[b]

                    if t == LAST or l == L - 1:
                        # final layer: jb-halves, row-space bias+relu,
                        # ship row-contiguous on the scalar ring.
                        prow = psr.tile([1, D], F32)
                        for jb in range(2):
                            for ki in range(KI):
                                if t == LAST:
                                    blk = wtiles[t][jb * 4 + ki // 2]
                                    rhs = blk[:, (ki % 2) * 512:
                                              (ki % 2) * 512 + 512]
                                else:
                                    wh = (wtiles[t][0] if ki < KH
                                          else wtiles[t][1])
                                    rhs = wh[:, (ki % KH) * D + jb * 512:
                                              (ki % KH) * D + jb * 512 + 512]
                                nc.tensor.matmul(
                                    prow[0:1, jb * 512:(jb + 1) * 512],
                                    lhsT=cur[:, ki:ki + 1],
                                    rhs=rhs,
                                    start=(ki == 0),
                                    stop=(ki == KI - 1),
                                )
                        orow = hrp.tile([1, D], F32, tag="orow")
                        for jb in range(2):
                            sl = slice(jb * 512, (jb + 1) * 512)
                            zrow = hrp.tile([1, 512], F32, tag="zrow")
                            nc.vector.tensor_add(
                                out=zrow,
                                in0=prow[0:1, sl],
                                in1=bmrt[0:1, b * 2 * D + jb * 512:
                                         b * 2 * D + (jb + 1) * 512],
                            )
                            trow = hrp.tile([1, 512], F32, tag="trow")
                            nc.vector.scalar_tensor_tensor(
                                out=trow,
                                in0=zrow,
                                scalar=0.0,
                                in1=bmrt[0:1, b * 2 * D + D + jb * 512:
                                         b * 2 * D + D + (jb + 1) * 512],
                                op0=mybir.AluOpType.min,
                                op1=mybir.AluOpType.mult,
                            )
                            nc.vector.tensor_sub(
                                out=orow[0:1, sl], in0=zrow, in1=trow)
                        nc.scalar.dma_start(
                            out=out[b:b + 1, :], in_=orow)
                        continue

                    # z = h @ W in 4 independent quarter ranges of 256 so
                    # each quarter's chain starts as soon as it closes.
                    prow = psr.tile([1, D], F32)
                    hrow = hrp.tile([1, D], F32)
                    pcol = hcp.tile([P, KI], F32, tag="pcol")
                    for q in range(NQ):
                        for ki in range(KI):
                            wh = wtiles[t][0] if ki < KH else wtiles[t][1]
                            base = (ki % KH) * D + q * QW
                            nc.tensor.matmul(
                                prow[0:1, q * QW:(q + 1) * QW],
                                lhsT=cur[:, ki:ki + 1],
                                rhs=wh[:, base:base + QW],
                                start=(ki == 0),
                                stop=(ki == KI - 1),
                            )
                        # bias-fused PSUM->SBUF copy (row space), then the
                        # [32, 8] scatter to column layout on the scalar
                        # HWDGE ring (it carries nothing else, so the
                        # scatter drains in ~one round-robin packet).
                        nc.vector.tensor_add(
                            out=hrow[0:1, q * QW:(q + 1) * QW],
                            in0=prow[0:1, q * QW:(q + 1) * QW],
                            in1=brt[0:1, t * D + q * QW:t * D + (q + 1) * QW],
                        )
                        nc.scalar.dma_start(
                            out=pcol[q * 32:(q + 1) * 32, :],
                            in_=hrow[0:1, q * QW:(q + 1) * QW].rearrange(
                                "o (p k) -> o p k", k=KI),
                        )

                    # masked relu per quarter: h = zb - mask*min(zb, 0)
                    tmp = hcp.tile([P, KI], F32, tag="tmp")
                    hnew = hcp.tile([P, KI], hdt, tag="h")
                    for q in range(NQ):
                        ps = slice(q * 32, (q + 1) * 32)
                        nc.vector.scalar_tensor_tensor(
                            out=tmp[ps, :],
                            in0=pcol[ps, :],
                            scalar=0.0,
                            in1=mkt[ps, t * KI:(t + 1) * KI],
                            op0=mybir.AluOpType.min,
                            op1=mybir.AluOpType.mult,
                        )
                        nc.vector.tensor_sub(
                            out=hnew[ps, :], in0=pcol[ps, :], in1=tmp[ps, :])
                    h[b] = hnew
    nc.finalize()
    return nc


def _get_nc():
    if WMODE not in _CACHE:
        _CACHE[WMODE] = _build(WMODE)
    return _CACHE[WMODE]


def _prep_core_inputs(c, x, weights, biases, masks):
    b0 = c * BC
    # weights[l, b, i, j], i = p*KI + ki  ->  [t, p, ki*1024 + j]
    # p-outer chunking: a pure reshape
    wc = np.ascontiguousarray(weights[:, b0:b0 + BC]).reshape(NT, P, KI * D)
    # last tile jb-major: [p, ki*1024 + jb*512 + j'] -> [p, jb*4096 + ki*512 + j']
    wl = wc[NT - 1].reshape(P, KI, 2, 512).transpose(0, 2, 1, 3)
    wc[NT - 1] = np.ascontiguousarray(wl).reshape(P, KI * D)
    if WMODE == "bf16":
        import ml_dtypes
        wc = wc.astype(ml_dtypes.bfloat16)
    # x[b, p*KI+k] -> [p, b*KI + k]
    xc = x[b0:b0 + BC].reshape(BC, P, KI)
    xc = np.ascontiguousarray(xc.transpose(1, 0, 2)).reshape(P, BC * KI)
    if WMODE == "bf16":
        import ml_dtypes
        xc = xc.astype(ml_dtypes.bfloat16)
    # bias rows: [1, t*D + j]
    import ml_dtypes
    brc = np.ascontiguousarray(
        biases[:, b0:b0 + BC]).reshape(1, NT * D).astype(ml_dtypes.bfloat16)
    # masks column layout: [p, t*KI + ki], i = p*KI + ki
    mc = masks[:, b0:b0 + BC].astype(np.float32).reshape(L, BC, P, KI)
    mc = np.ascontiguousarray(mc.transpose(2, 0, 1, 3)).reshape(P, NT * KI)
    # last layer's bias/mask, row-major per sample: [b, (bias | mask)]
    bmrc = np.concatenate(
        [biases[L - 1, b0:b0 + BC],
         masks[L - 1, b0:b0 + BC].astype(np.float32)],
        axis=1,
    )
    bmrc = np.ascontiguousarray(bmrc)
    return {"w": wc, "x": xc, "br": brc, "mk": mc, "bmr": bmrc}


def _run(inputs: dict, trace: bool = False, trace_cores=None, tmpdir=None):
    x = np.asarray(inputs["x"], dtype=np.float32)
    weights = np.asarray(inputs["weights"], dtype=np.float32)
    biases = np.asarray(inputs["biases"], dtype=np.float32)
    masks = np.asarray(inputs["masks"])

    nc = _get_nc()
    in_maps = [
        _prep_core_inputs(c, x, weights, biases, masks) for c in range(NCORES)
    ]
    kw = {}
    if trace_cores is not None:
        kw["trace_cores"] = trace_cores
    if tmpdir is not None:
        kw["tmpdir"] = tmpdir
    res = run_bass_kernel_spmd(
        nc, in_maps, core_ids=list(range(NCORES)), trace=trace, **kw
    )
    outs = []
    for c in range(NCORES):
        oc = res.results[c]["out"]  # [BC, D] row-major
        outs.append(oc)
    full = np.concatenate(outs, axis=0).astype(np.float32)
    return full, res


def kernel(**inputs) -> np.ndarray:
    full, _ = _run(inputs, trace=False)
    return full


# revision 16
# speedup vs baseline: 1.2304x; 1.2304x over previous
"""Trainium2 Bass kernel for NeuralDisCoCirc forward pass.

Problem: L=8 sequential layers; each layer, per sample b:
    z = h @ W[l,b] + bias[l,b];  h = where(mask[l,b], relu(z), z)
Shapes: x [16,1024] f32, weights [8,16,1024,1024] f32,
        biases/masks [8,16,1024].

Strategy (data-parallel over batch, 2 samples per core, 8 cores):
  - Weights stream as bf16 (host cast; rel err ~4e-3 over 8 layers,
    well under the 2e-2 gate) on the SINGLE sync HWDGE ring: per-layer
    tiles are [128 x 8KB] contiguous rows, 8-deep prefetch so the
    stream never stalls on pool-slot release.  The scalar HWDGE ring
    carries ONLY the small row->column scatters + final outputs, so
    they round-robin against the weight queue at packet granularity
    (~0.5us) instead of queueing behind it (~3.5us on SWDGE).
  - h is kept column-major ([p, c], element i = p*KI + c) as the
    matmul stationary operand.  Per layer: 32 accumulating matmuls
    (h chunk [128,1] stationary, W chunk [128,256] moving) close z in
    QUARTER ranges, so each quarter's chain (bias-fused PSUM->SBUF row
    copy on DVE -> [32,8] scatter on scalar ring -> masked-relu on the
    column quarter) pipelines under the remaining matmuls.  Only the
    last quarter's short chain (~1.5-2.5us) separates layer l from
    layer l+2 (same sample), under the ~3.4us the other sample's
    matmul group provides - so the PE stays fed and HAM stays warm.
  - bias is added in ROW space during the PSUM->SBUF copy (the copy
    becomes a tensor_add), so the column side needs only 2 DVE ops:
    h = zb - mask*min(zb, 0).
  - The final layer skips the column layout: bias + masked relu run in
    row layout per jb half and the output ships row-contiguous on the
    scalar ring.
  - Memory-bound: 32 MB of bf16 weights per core at ~358 GB/s/core.
"""

import numpy as np

import concourse.bass as bass
import concourse.mybir as mybir
from concourse import bacc
from concourse.tile import TileContext
from concourse.bass_utils import run_bass_kernel_spmd

L = 8          # layers
B = 16         # full batch
D = 1024       # width
NCORES = 8
BC = B // NCORES   # samples per core (2)
NT = L * BC        # (layer, sample) tiles per core (16)
KI = D // 128      # 8 chunks of 128 along the contraction dim
P = 128
NQ = 4             # z closes in quarters of 256
QW = D // NQ       # 256

F32 = mybir.dt.float32
F32R = mybir.dt.float32r
BF16 = mybir.dt.bfloat16

WMODE = "bf16"

_CACHE = {}


def _build(wmode: str) -> bass.Bass:
    wdt = {"bf16": BF16, "f32r": F32R, "f32": F32}[wmode]
    hdt = {"bf16": BF16, "f32r": F32R, "f32": F32}[wmode]

    nc = bacc.Bacc("TRN2", target_bir_lowering=False, debug=False)
    w = nc.declare_dram_parameter("w", [NT, P, KI * D], wdt, isOutput=False)
    x = nc.declare_dram_parameter("x", [P, BC * KI], hdt, isOutput=False)
    # bias rows for all tiles (bf16 to halve SBUF footprint); masks col layout
    br = nc.declare_dram_parameter("br", [1, NT * D], BF16, isOutput=False)
    mk = nc.declare_dram_parameter("mk", [P, NT * KI], F32, isOutput=False)
    # last layer's bias/mask in ROW layout: bmr[b] = [bias_row | mask_row]
    bmr = nc.declare_dram_parameter("bmr", [BC, 2 * D], F32, isOutput=False)
    out = nc.declare_dram_parameter("out", [BC, D], F32, isOutput=True)

    with TileContext(nc) as tc:
        with (
            tc.tile_pool(name="wp", bufs=8) as wp,
            tc.tile_pool(name="const", bufs=1) as cp,
            tc.tile_pool(name="hrow", bufs=2) as hrp,
            tc.tile_pool(name="hcol", bufs=4) as hcp,
            tc.tile_pool(name="psr", bufs=4, space="PSUM") as psr,
        ):
            # Small SWDGE input loads first: they ride their own engine
            # (gpsimd) and land while the HWDGE weight flood is still in
            # descriptor generation.
            brt = cp.tile([1, NT * D], BF16, tag="br")
            mkt = cp.tile([P, NT * KI], F32, tag="mk")
            bmrt = cp.tile([1, BC * 2 * D], F32, tag="bmr")
            xt = cp.tile([P, BC * KI], hdt, tag="x")
            nc.gpsimd.dma_start(out=xt, in_=x[:])
            nc.gpsimd.dma_start(out=brt, in_=br[:])
            nc.gpsimd.dma_start(out=mkt, in_=mk[:])
            nc.gpsimd.dma_start(
                out=bmrt, in_=bmr[:].rearrange("b n -> () (b n)"))

            KH = KI // 2  # ki chunks per half-tile
            LAST = NT - 1
            wtiles = {}
            for t in range(NT):
                if t == 0:
                    # small starter blocks so the first weight bytes land
                    # earlier (fewer descriptors before the ring fires)
                    ST = D
                    wa = wp.tile([P, KH * D], wdt, tag="wa")
                    wb = wp.tile([P, KH * D], wdt, tag="wb")
                    nc.sync.dma_start(out=wa[:, :ST], in_=w[t, :, :ST])
                    nc.sync.dma_start(
                        out=wb[:, :ST], in_=w[t, :, KH * D:KH * D + ST])
                    nc.sync.dma_start(out=wa[:, ST:], in_=w[t, :, ST:KH * D])
                    nc.sync.dma_start(
                        out=wb[:, ST:], in_=w[t, :, KH * D + ST:])
                    wtiles[t] = (wa, wb)
                elif t < LAST:
                    wa = wp.tile([P, KH * D], wdt, tag="wa")
                    wb = wp.tile([P, KH * D], wdt, tag="wb")
                    nc.sync.dma_start(out=wa, in_=w[t, :, : KH * D])
                    nc.sync.dma_start(out=wb, in_=w[t, :, KH * D:])
                    wtiles[t] = (wa, wb)
                else:
                    # last tile: host re-laid it jb-major
                    # ([p, jb*4096 + ki*512 + j']), streamed as 8
                    # contiguous blocks so the jb0 PSUM group closes at
                    # the tile's halfway point.
                    qs = []
                    for q in range(8):
                        wq = wp.tile([P, KH * D], wdt,
                                     tag=("wa" if q % 2 == 0 else "wb"))
                        nc.sync.dma_start(
                            out=wq[:, :D],
                            in_=w[t, :, q * D:(q + 1) * D],
                        )
                        qs.append(wq[:, :D])
                    wtiles[t] = tuple(qs)

            h = [xt[:, b * KI:(b + 1) * KI] for b in range(BC)]

            for l in range(L):
                for b in range(BC):
                    t = l * BC + b
                    cur = h[b]

                    if t == LAST or l == L - 1:
                        # final layer: jb-halves, row-space bias+relu,
                        # ship row-contiguous on the scalar ring.
                        prow = psr.tile([1, D], F32)
                        for jb in range(2):
                            for ki in range(KI):
                                if t == LAST:
                                    blk = wtiles[t][jb * 4 + ki // 2]
                                    rhs = blk[:, (ki % 2) * 512:
                                              (ki % 2) * 512 + 512]
                                else:
                                    wh = (wtiles[t][0] if ki < KH
                                          else wtiles[t][1])
                                    rhs = wh[:, (ki % KH) * D + jb * 512:
                                              (ki % KH) * D + jb * 512 + 512]
                                nc.tensor.matmul(
                                    prow[0:1, jb * 512:(jb + 1) * 512],
                                    lhsT=cur[:, ki:ki + 1],
                                    rhs=rhs,
                                    start=(ki == 0),
                                    stop=(ki == KI - 1),
                                )
                        orow = hrp.tile([1, D], F32, tag="orow")
                        for jb in range(2):
                            sl = slice(jb * 512, (jb + 1) * 512)
                            zrow = hrp.tile([1, 512], F32, tag="zrow")
                            nc.vector.tensor_add(
                                out=zrow,
                                in0=prow[0:1, sl],
                                in1=bmrt[0:1, b * 2 * D + jb * 512:
                                         b * 2 * D + (jb + 1) * 512],
                            )
                            trow = hrp.tile([1, 512], F32, tag="trow")
                            nc.vector.scalar_tensor_tensor(
                                out=trow,
                                in0=zrow,
                                scalar=0.0,
                                in1=bmrt[0:1, b * 2 * D + D + jb * 512:
                                         b * 2 * D + D + (jb + 1) * 512],
                                op0=mybir.AluOpType.min,
                                op1=mybir.AluOpType.mult,
                            )
                            nc.vector.tensor_sub(
                                out=orow[0:1, sl], in0=zrow, in1=trow)
                        nc.scalar.dma_start(
                            out=out[b:b + 1, :], in_=orow)
                        continue

                    # z = h @ W in 2 half ranges of 512 (one PSUM bank
                    # each, so the ranges never serialize against the
                    # other half's DVE drain).
                    prow = psr.tile([1, D], F32)
                    hrow = hrp.tile([1, D], F32)
                    pcol = hcp.tile([P, KI], F32, tag="pcol")
                    for jb in range(2):
                        for ki in range(KI):
                            wh = wtiles[t][0] if ki < KH else wtiles[t][1]
                            base = (ki % KH) * D + jb * 512
                            nc.tensor.matmul(
                                prow[0:1, jb * 512:(jb + 1) * 512],
                                lhsT=cur[:, ki:ki + 1],
                                rhs=wh[:, base:base + 512],
                                start=(ki == 0),
                                stop=(ki == KI - 1),
                            )
                        # bias-fused PSUM->SBUF copy (row space), then the
                        # [64, 8] scatter to column layout on the scalar
                        # HWDGE ring (it carries nothing else, so the
                        # scatter drains in ~one round-robin packet).
                        nc.vector.tensor_add(
                            out=hrow[0:1, jb * 512:(jb + 1) * 512],
                            in0=prow[0:1, jb * 512:(jb + 1) * 512],
                            in1=brt[0:1, t * D + jb * 512:
                                     t * D + (jb + 1) * 512],
                        )
                        nc.scalar.dma_start(
                            out=pcol[jb * 64:(jb + 1) * 64, :],
                            in_=hrow[0:1, jb * 512:(jb + 1) * 512].rearrange(
                                "o (p k) -> o p k", k=KI),
                        )

                    # masked relu per half: h = zb - mask*min(zb, 0)
                    tmp = hcp.tile([P, KI], F32, tag="tmp")
                    hnew = hcp.tile([P, KI], hdt, tag="h")
                    for jb in range(2):
                        ps = slice(jb * 64, (jb + 1) * 64)
                        nc.vector.scalar_tensor_tensor(
                            out=tmp[ps, :],
                            in0=pcol[ps, :],
                            scalar=0.0,
                            in1=mkt[ps, t * KI:(t + 1) * KI],
                            op0=mybir.AluOpType.min,
                            op1=mybir.AluOpType.mult,
                        )
                        nc.vector.tensor_sub(
                            out=hnew[ps, :], in0=pcol[ps, :], in1=tmp[ps, :])
                    h[b] = hnew
    nc.finalize()
    return nc


def _get_nc():
    if WMODE not in _CACHE:
        _CACHE[WMODE] = _build(WMODE)
    return _CACHE[WMODE]


def _prep_core_inputs(c, x, weights, biases, masks):
    b0 = c * BC
    # weights[l, b, i, j], i = p*KI + ki  ->  [t, p, ki*1024 + j]
    # p-outer chunking: a pure reshape
    wc = np.ascontiguousarray(weights[:, b0:b0 + BC]).reshape(NT, P, KI * D)
    # last tile jb-major: [p, ki*1024 + jb*512 + j'] -> [p, jb*4096 + ki*512 + j']
    wl = wc[NT - 1].reshape(P, KI, 2, 512).transpose(0, 2, 1, 3)
    wc[NT - 1] = np.ascontiguousarray(wl).reshape(P, KI * D)
    if WMODE == "bf16":
        import ml_dtypes
        wc = wc.astype(ml_dtypes.bfloat16)
    # x[b, p*KI+k] -> [p, b*KI + k]
    xc = x[b0:b0 + BC].reshape(BC, P, KI)
    xc = np.ascontiguousarray(xc.transpose(1, 0, 2)).reshape(P, BC * KI)
    if WMODE == "bf16":
        import ml_dtypes
        xc = xc.astype(ml_dtypes.bfloat16)
    # bias rows: [1, t*D + j]
    import ml_dtypes
    brc = np.ascontiguousarray(
        biases[:, b0:b0 + BC]).reshape(1, NT * D).astype(ml_dtypes.bfloat16)
    # masks column layout: [p, t*KI + ki], i = p*KI + ki
    mc = masks[:, b0:b0 + BC].astype(np.float32).reshape(L, BC, P, KI)
    mc = np.ascontiguousarray(mc.transpose(2, 0, 1, 3)).reshape(P, NT * KI)
    # last layer's bias/mask, row-major per sample: [b, (bias | mask)]
    bmrc = np.concatenate(
        [biases[L - 1, b0:b0 + BC],
         masks[L - 1, b0:b0 + BC].astype(np.float32)],
        axis=1,
    )
    bmrc = np.ascontiguousarray(bmrc)
    return {"w": wc, "x": xc, "br": brc, "mk": mc, "bmr": bmrc}


def _run(inputs: dict, trace: bool = False, trace_cores=None, tmpdir=None):
    x = np.asarray(inputs["x"], dtype=np.float32)
    weights = np.asarray(inputs["weights"], dtype=np.float32)
    biases = np.asarray(inputs["biases"], dtype=np.float32)
    masks = np.asarray(inputs["masks"])

    nc = _get_nc()
    in_maps = [
        _prep_core_inputs(c, x, weights, biases, masks) for c in range(NCORES)
    ]
    kw = {}
    if trace_cores is not None:
        kw["trace_cores"] = trace_cores
    if tmpdir is not None:
        kw["tmpdir"] = tmpdir
    res = run_bass_kernel_spmd(
        nc, in_maps, core_ids=list(range(NCORES)), trace=trace, **kw
    )
    outs = []
    for c in range(NCORES):
        oc = res.results[c]["out"]  # [BC, D] row-major
        outs.append(oc)
    full = np.concatenate(outs, axis=0).astype(np.float32)
    return full, res


def kernel(**inputs) -> np.ndarray:
    full, _ = _run(inputs, trace=False)
    return full


# revision 18
# speedup vs baseline: 1.2522x; 1.0177x over previous
"""Trainium2 Bass kernel for NeuralDisCoCirc forward pass.

Problem: L=8 sequential layers; each layer, per sample b:
    z = h @ W[l,b] + bias[l,b];  h = where(mask[l,b], relu(z), z)
Shapes: x [16,1024] f32, weights [8,16,1024,1024] f32,
        biases/masks [8,16,1024].

Strategy (data-parallel over batch, 2 samples per core, 8 cores):
  - Weights stream as bf16 (host cast; rel err ~4e-3 over 8 layers,
    well under the 2e-2 gate) on the SINGLE sync HWDGE ring: per-layer
    tiles are [128 x 8KB] contiguous rows, 8-deep prefetch so the
    stream never stalls on pool-slot release.  The scalar HWDGE ring
    carries ONLY the small row->column scatters + final outputs, so
    they round-robin against the weight queue at packet granularity
    (~0.5us) instead of queueing behind it (~3.5us on SWDGE).
  - h is kept column-major ([p, c], element i = p*KI + c) as the
    matmul stationary operand.  Per layer: 32 accumulating matmuls
    (h chunk [128,1] stationary, W chunk [128,256] moving) close z in
    QUARTER ranges, so each quarter's chain (bias-fused PSUM->SBUF row
    copy on DVE -> [32,8] scatter on scalar ring -> masked-relu on the
    column quarter) pipelines under the remaining matmuls.  Only the
    last quarter's short chain (~1.5-2.5us) separates layer l from
    layer l+2 (same sample), under the ~3.4us the other sample's
    matmul group provides - so the PE stays fed and HAM stays warm.
  - bias is added in ROW space during the PSUM->SBUF copy (the copy
    becomes a tensor_add), so the column side needs only 2 DVE ops:
    h = zb - mask*min(zb, 0).
  - The final layer skips the column layout: bias + masked relu run in
    row layout per jb half and the output ships row-contiguous on the
    scalar ring.
  - Memory-bound: 32 MB of bf16 weights per core at ~358 GB/s/core.
"""

import numpy as np

import concourse.bass as bass
import concourse.mybir as mybir
from concourse import bacc
from concourse.tile import TileContext
from concourse.bass_utils import run_bass_kernel_spmd

L = 8          # layers
B = 16         # full batch
D = 1024       # width
NCORES = 8
BC = B // NCORES   # samples per core (2)
NT = L * BC        # (layer, sample) tiles per core (16)
KI = D // 128      # 8 chunks of 128 along the contraction dim
P = 128
NQ = 4             # z closes in quarters of 256
QW = D // NQ       # 256

F32 = mybir.dt.float32
F32R = mybir.dt.float32r
BF16 = mybir.dt.bfloat16

WMODE = "bf16"

_CACHE = {}


def _build(wmode: str) -> bass.Bass:
    wdt = {"bf16": BF16, "f32r": F32R, "f32": F32}[wmode]
    hdt = {"bf16": BF16, "f32r": F32R, "f32": F32}[wmode]

    nc = bacc.Bacc("TRN2", target_bir_lowering=False, debug=False)
    w = nc.declare_dram_parameter("w", [NT, P, KI * D], wdt, isOutput=False)
    x = nc.declare_dram_parameter("x", [P, BC * KI], hdt, isOutput=False)
    # bias rows for all tiles (bf16 to halve SBUF footprint); masks col layout
    br = nc.declare_dram_parameter("br", [1, NT * D], BF16, isOutput=False)
    mk = nc.declare_dram_parameter("mk", [P, NT * KI], F32, isOutput=False)
    # last layer's bias/mask in ROW layout: bmr[b] = [bias_row | mask_row]
    bmr = nc.declare_dram_parameter("bmr", [BC, 2 * D], F32, isOutput=False)
    out = nc.declare_dram_parameter("out", [BC, D], F32, isOutput=True)

    with TileContext(nc) as tc:
        with (
            tc.tile_pool(name="wp", bufs=7) as wp,
            tc.tile_pool(name="wl", bufs=1) as wlp,
            tc.tile_pool(name="const", bufs=1) as cp,
            tc.tile_pool(name="hrow", bufs=2) as hrp,
            tc.tile_pool(name="hcol", bufs=4) as hcp,
            tc.tile_pool(name="psr", bufs=4, space="PSUM") as psr,
        ):
            # Small SWDGE input loads first: they ride their own engine
            # (gpsimd) and land while the HWDGE weight flood is still in
            # descriptor generation.
            brt = cp.tile([1, NT * D], BF16, tag="br")
            mkt = cp.tile([P, NT * KI], F32, tag="mk")
            bmrt = cp.tile([1, BC * 2 * D], F32, tag="bmr")
            xt = cp.tile([P, BC * KI], hdt, tag="x")
            nc.gpsimd.dma_start(out=xt, in_=x[:])
            nc.gpsimd.dma_start(out=brt, in_=br[:])
            nc.gpsimd.dma_start(out=mkt, in_=mk[:])
            nc.gpsimd.dma_start(
                out=bmrt, in_=bmr[:].rearrange("b n -> () (b n)"))

            KH = KI // 2  # ki chunks per half-tile
            LAST = NT - 1
            wtiles = {}
            for t in range(NT):
                if t == 0:
                    # small starter blocks so the first weight bytes land
                    # earlier (fewer descriptors before the ring fires)
                    ST = D
                    wa = wp.tile([P, KH * D], wdt, tag="wa")
                    wb = wp.tile([P, KH * D], wdt, tag="wb")
                    nc.sync.dma_start(out=wa[:, :ST], in_=w[t, :, :ST])
                    nc.sync.dma_start(
                        out=wb[:, :ST], in_=w[t, :, KH * D:KH * D + ST])
                    nc.sync.dma_start(out=wa[:, ST:], in_=w[t, :, ST:KH * D])
                    nc.sync.dma_start(
                        out=wb[:, ST:], in_=w[t, :, KH * D + ST:])
                    wtiles[t] = (wa, wb)
                elif t < LAST:
                    wa = wp.tile([P, KH * D], wdt, tag="wa")
                    wb = wp.tile([P, KH * D], wdt, tag="wb")
                    nc.sync.dma_start(out=wa, in_=w[t, :, : KH * D])
                    nc.sync.dma_start(out=wb, in_=w[t, :, KH * D:])
                    wtiles[t] = (wa, wb)
                else:
                    # last tile: host re-laid it jb-major
                    # ([p, jb*4096 + ki*512 + j']), streamed as 8
                    # contiguous blocks with DEDICATED slots (no pool
                    # cycling), so their DMAs issue with no slot wait and
                    # the ring never stalls at the stream's end.
                    qs = []
                    for q in range(8):
                        wq = wlp.tile([P, D], wdt, tag=f"wl{q}")
                        nc.sync.dma_start(
                            out=wq,
                            in_=w[t, :, q * D:(q + 1) * D],
                        )
                        qs.append(wq)
                    wtiles[t] = tuple(qs)

            h = [xt[:, b * KI:(b + 1) * KI] for b in range(BC)]

            for l in range(L):
                for b in range(BC):
                    t = l * BC + b
                    cur = h[b]

                    if t == LAST or l == L - 1:
                        # final layer: jb-halves, row-space bias+relu,
                        # ship row-contiguous on the scalar ring.
                        prow = psr.tile([1, D], F32)
                        for jb in range(2):
                            for ki in range(KI):
                                if t == LAST:
                                    blk = wtiles[t][jb * 4 + ki // 2]
                                    rhs = blk[:, (ki % 2) * 512:
                                              (ki % 2) * 512 + 512]
                                else:
                                    wh = (wtiles[t][0] if ki < KH
                                          else wtiles[t][1])
                                    rhs = wh[:, (ki % KH) * D + jb * 512:
                                              (ki % KH) * D + jb * 512 + 512]
                                nc.tensor.matmul(
                                    prow[0:1, jb * 512:(jb + 1) * 512],
                                    lhsT=cur[:, ki:ki + 1],
                                    rhs=rhs,
                                    start=(ki == 0),
                                    stop=(ki == KI - 1),
                                )
                        orow = hrp.tile([1, D], F32, tag="orow")
                        for jb in range(2):
                            sl = slice(jb * 512, (jb + 1) * 512)
                            zrow = hrp.tile([1, 512], F32, tag="zrow")
                            nc.vector.tensor_add(
                                out=zrow,
                                in0=prow[0:1, sl],
                                in1=bmrt[0:1, b * 2 * D + jb * 512:
                                         b * 2 * D + (jb + 1) * 512],
                            )
                            trow = hrp.tile([1, 512], F32, tag="trow")
                            nc.vector.scalar_tensor_tensor(
                                out=trow,
                                in0=zrow,
                                scalar=0.0,
                                in1=bmrt[0:1, b * 2 * D + D + jb * 512:
                                         b * 2 * D + D + (jb + 1) * 512],
                                op0=mybir.AluOpType.min,
                                op1=mybir.AluOpType.mult,
                            )
                            nc.vector.tensor_sub(
                                out=orow[0:1, sl], in0=zrow, in1=trow)
                        nc.scalar.dma_start(
                            out=out[b:b + 1, :], in_=orow)
                        continue

                    # z = h @ W in 2 half ranges of 512 (one PSUM bank
                    # each, so the ranges never serialize against the
                    # other half's DVE drain).
                    prow = psr.tile([1, D], F32)
                    hrow = hrp.tile([1, D], F32)
                    pcol = hcp.tile([P, KI], F32, tag="pcol")
                    for jb in range(2):
                        for ki in range(KI):
                            wh = wtiles[t][0] if ki < KH else wtiles[t][1]
                            base = (ki % KH) * D + jb * 512
                            nc.tensor.matmul(
                                prow[0:1, jb * 512:(jb + 1) * 512],
                                lhsT=cur[:, ki:ki + 1],
                                rhs=wh[:, base:base + 512],
                                start=(ki == 0),
                                stop=(ki == KI - 1),
                            )
                        # bias-fused PSUM->SBUF copy (row space), then the
                        # [64, 8] scatter to column layout on the scalar
                        # HWDGE ring (it carries nothing else, so the
                        # scatter drains in ~one round-robin packet).
                        nc.vector.tensor_add(
                            out=hrow[0:1, jb * 512:(jb + 1) * 512],
                            in0=prow[0:1, jb * 512:(jb + 1) * 512],
                            in1=brt[0:1, t * D + jb * 512:
                                     t * D + (jb + 1) * 512],
                        )
                        nc.scalar.dma_start(
                            out=pcol[jb * 64:(jb + 1) * 64, :],
                            in_=hrow[0:1, jb * 512:(jb + 1) * 512].rearrange(
                                "o (p k) -> o p k", k=KI),
                        )

                    # masked relu per half: h = zb - mask*min(zb, 0)
                    tmp = hcp.tile([P, KI], F32, tag="tmp")
                    hnew = hcp.tile([P, KI], hdt, tag="h")
                    for jb in range(2):
                        ps = slice(jb * 64, (jb + 1) * 64)
                        nc.vector.scalar_tensor_tensor(
                            out=tmp[ps, :],
                            in0=pcol[ps, :],
                            scalar=0.0,
                            in1=mkt[ps, t * KI:(t + 1) * KI],
                            op0=mybir.AluOpType.min,
                            op1=mybir.AluOpType.mult,
                        )
                        nc.vector.tensor_sub(
                            out=hnew[ps, :], in0=pcol[ps, :], in1=tmp[ps, :])
                    h[b] = hnew
    nc.finalize()
    return nc


def _get_nc():
    if WMODE not in _CACHE:
        _CACHE[WMODE] = _build(WMODE)
    return _CACHE[WMODE]


def _prep_core_inputs(c, x, weights, biases, masks):
    b0 = c * BC
    # weights[l, b, i, j], i = p*KI + ki  ->  [t, p, ki*1024 + j]
    # p-outer chunking: a pure reshape
    wc = np.ascontiguousarray(weights[:, b0:b0 + BC]).reshape(NT, P, KI * D)
    # last tile jb-major: [p, ki*1024 + jb*512 + j'] -> [p, jb*4096 + ki*512 + j']
    wl = wc[NT - 1].reshape(P, KI, 2, 512).transpose(0, 2, 1, 3)
    wc[NT - 1] = np.ascontiguousarray(wl).reshape(P, KI * D)
    if WMODE == "bf16":
        import ml_dtypes
        wc = wc.astype(ml_dtypes.bfloat16)
    # x[b, p*KI+k] -> [p, b*KI + k]
    xc = x[b0:b0 + BC].reshape(BC, P, KI)
    xc = np.ascontiguousarray(xc.transpose(1, 0, 2)).reshape(P, BC * KI)
    if WMODE == "bf16":
        import ml_dtypes
        xc = xc.astype(ml_dtypes.bfloat16)
    # bias rows: [1, t*D + j]
    import ml_dtypes
    brc = np.ascontiguousarray(
        biases[:, b0:b0 + BC]).reshape(1, NT * D).astype(ml_dtypes.bfloat16)
    # masks column layout: [p, t*KI + ki], i = p*KI + ki
    mc = masks[:, b0:b0 + BC].astype(np.float32).reshape(L, BC, P, KI)
    mc = np.ascontiguousarray(mc.transpose(2, 0, 1, 3)).reshape(P, NT * KI)
    # last layer's bias/mask, row-major per sample: [b, (bias | mask)]
    bmrc = np.concatenate(
        [biases[L - 1, b0:b0 + BC],
         masks[L - 1, b0:b0 + BC].astype(np.float32)],
        axis=1,
    )
    bmrc = np.ascontiguousarray(bmrc)
    return {"w": wc, "x": xc, "br": brc, "mk": mc, "bmr": bmrc}


def _run(inputs: dict, trace: bool = False, trace_cores=None, tmpdir=None):
    x = np.asarray(inputs["x"], dtype=np.float32)
    weights = np.asarray(inputs["weights"], dtype=np.float32)
    biases = np.asarray(inputs["biases"], dtype=np.float32)
    masks = np.asarray(inputs["masks"])

    nc = _get_nc()
    in_maps = [
        _prep_core_inputs(c, x, weights, biases, masks) for c in range(NCORES)
    ]
    kw = {}
    if trace_cores is not None:
        kw["trace_cores"] = trace_cores
    if tmpdir is not None:
        kw["tmpdir"] = tmpdir
    res = run_bass_kernel_spmd(
        nc, in_maps, core_ids=list(range(NCORES)), trace=trace, **kw
    )
    outs = []
    for c in range(NCORES):
        oc = res.results[c]["out"]  # [BC, D] row-major
        outs.append(oc)
    full = np.concatenate(outs, axis=0).astype(np.float32)
    return full, res


def kernel(**inputs) -> np.ndarray:
    full, _ = _run(inputs, trace=False)
    return full


# revision 26
# speedup vs baseline: 1.3646x; 1.0897x over previous
"""Trainium2 Bass kernel for NeuralDisCoCirc forward pass.

Problem: L=8 sequential layers; each layer, per sample b:
    z = h @ W[l,b] + bias[l,b];  h = where(mask[l,b], relu(z), z)
Shapes: x [16,1024] f32, weights [8,16,1024,1024] f32,
        biases/masks [8,16,1024].

Strategy (data-parallel over batch, 2 samples per core, 8 cores):
  - Weights stream as bf16 (host cast; rel err ~4e-3 over 8 layers,
    well under the 2e-2 gate) on the SINGLE sync HWDGE ring: per-layer
    tiles are [128 x 8KB] contiguous rows, 8-deep prefetch so the
    stream never stalls on pool-slot release.  The scalar HWDGE ring
    carries ONLY the small row->column scatters + final outputs, so
    they round-robin against the weight queue at packet granularity
    (~0.5us) instead of queueing behind it (~3.5us on SWDGE).
  - h is kept column-major ([p, c], element i = p*KI + c) as the
    matmul stationary operand.  Per layer: 32 accumulating matmuls
    (h chunk [128,1] stationary, W chunk [128,256] moving) close z in
    QUARTER ranges, so each quarter's chain (bias-fused PSUM->SBUF row
    copy on DVE -> [32,8] scatter on scalar ring -> masked-relu on the
    column quarter) pipelines under the remaining matmuls.  Only the
    last quarter's short chain (~1.5-2.5us) separates layer l from
    layer l+2 (same sample), under the ~3.4us the other sample's
    matmul group provides - so the PE stays fed and HAM stays warm.
  - bias is added in ROW space during the PSUM->SBUF copy (the copy
    becomes a tensor_add), so the column side needs only 2 DVE ops:
    h = zb - mask*min(zb, 0).
  - The final layer skips the column layout: bias + masked relu run in
    row layout per jb half and the output ships row-contiguous on the
    scalar ring.
  - Memory-bound: 32 MB of bf16 weights per core at ~358 GB/s/core.
"""

import numpy as np

import concourse.bass as bass
import concourse.mybir as mybir
from concourse import bacc
from concourse.tile import TileContext
from concourse.bass_utils import run_bass_kernel_spmd

L = 8          # layers
B = 16         # full batch
D = 1024       # width
NCORES = 8
BC = B // NCORES   # samples per core (2)
NT = L * BC        # (layer, sample) tiles per core (16)
KI = D // 128      # 8 chunks of 128 along the contraction dim
P = 128
NQ = 4             # z closes in quarters of 256
QW = D // NQ       # 256

F32 = mybir.dt.float32
F32R = mybir.dt.float32r
BF16 = mybir.dt.bfloat16

WMODE = "bf16"

_CACHE = {}


def _build(wmode: str) -> bass.Bass:
    wdt = {"bf16": BF16, "f32r": F32R, "f32": F32}[wmode]
    hdt = {"bf16": BF16, "f32r": F32R, "f32": F32}[wmode]

    nc = bacc.Bacc("TRN2", target_bir_lowering=False, debug=False)
    w = nc.declare_dram_parameter("w", [NT, P, KI * D], wdt, isOutput=False)
    x = nc.declare_dram_parameter("x", [P, BC * KI], hdt, isOutput=False)
    # bias rows for all tiles (bf16 to halve SBUF footprint); masks col layout
    br = nc.declare_dram_parameter("br", [1, NT * D], BF16, isOutput=False)
    mk = nc.declare_dram_parameter("mk", [P, NT * KI], F32, isOutput=False)
    # last layer's bias/mask in ROW layout: bmr[b] = [bias_row | mask_row]
    bmr = nc.declare_dram_parameter("bmr", [BC, 2 * D], F32, isOutput=False)
    ones = nc.declare_dram_parameter("ones", [1, 1], F32, isOutput=False)
    out = nc.declare_dram_parameter("out", [BC, D], F32, isOutput=True)

    with TileContext(nc) as tc:
        with (
            tc.tile_pool(name="wp", bufs=7) as wp,
            tc.tile_pool(name="wl", bufs=1) as wlp,
            tc.tile_pool(name="const", bufs=1) as cp,
            tc.tile_pool(name="hrow", bufs=2) as hrp,
            tc.tile_pool(name="hcol", bufs=4) as hcp,
            tc.tile_pool(name="psr", bufs=3, space="PSUM") as psr,
            tc.tile_pool(name="ptp", bufs=2, space="PSUM") as ptp,
        ):
            # Small SWDGE input loads first: they ride their own engine
            # (gpsimd) and land while the HWDGE weight flood is still in
            # descriptor generation.
            brt = cp.tile([1, NT * D], BF16, tag="br")
            mkt = cp.tile([P, NT * KI], F32, tag="mk")
            bmrt = cp.tile([1, BC * 2 * D], F32, tag="bmr")
            xt = cp.tile([P, BC * KI], hdt, tag="x")
            onet = cp.tile([1, 1], F32, tag="ones")
            nc.gpsimd.dma_start(out=xt, in_=x[:])
            nc.gpsimd.dma_start(out=onet, in_=ones[:])
            nc.gpsimd.dma_start(out=brt, in_=br[:])
            nc.gpsimd.dma_start(out=mkt, in_=mk[:])
            nc.gpsimd.dma_start(
                out=bmrt, in_=bmr[:].rearrange("b n -> () (b n)"))

            KH = KI // 2  # ki chunks per half-tile
            LAST = NT - 1
            wtiles = {}
            for t in range(NT):
                if t == 0:
                    # small starter blocks so the first weight bytes land
                    # earlier (fewer descriptors before the ring fires)
                    ST = D
                    wa = wp.tile([P, KH * D], wdt, tag="wa")
                    wb = wp.tile([P, KH * D], wdt, tag="wb")
                    nc.sync.dma_start(out=wa[:, :ST], in_=w[t, :, :ST])
                    nc.sync.dma_start(
                        out=wb[:, :ST], in_=w[t, :, KH * D:KH * D + ST])
                    nc.sync.dma_start(out=wa[:, ST:], in_=w[t, :, ST:KH * D])
                    nc.sync.dma_start(
                        out=wb[:, ST:], in_=w[t, :, KH * D + ST:])
                    wtiles[t] = (wa, wb)
                elif t < LAST:
                    wa = wp.tile([P, KH * D], wdt, tag="wa")
                    wb = wp.tile([P, KH * D], wdt, tag="wb")
                    nc.sync.dma_start(out=wa, in_=w[t, :, : KH * D])
                    nc.sync.dma_start(out=wb, in_=w[t, :, KH * D:])
                    wtiles[t] = (wa, wb)
                else:
                    # last tile: host re-laid it jb-major
                    # ([p, jb*4096 + ki*512 + j']), streamed as 8
                    # contiguous blocks with DEDICATED slots (no pool
                    # cycling), so their DMAs issue with no slot wait and
                    # the ring never stalls at the stream's end.
                    qs = []
                    for q in range(8):
                        wq = wlp.tile([P, D], wdt, tag=f"wl{q}")
                        nc.sync.dma_start(
                            out=wq,
                            in_=w[t, :, q * D:(q + 1) * D],
                        )
                        qs.append(wq)
                    wtiles[t] = tuple(qs)

            h = [xt[:, b * KI:(b + 1) * KI] for b in range(BC)]

            for l in range(L):
                for b in range(BC):
                    t = l * BC + b
                    cur = h[b]

                    if t == LAST or l == L - 1:
                        # final layer: jb-halves, row-space bias+relu,
                        # ship row-contiguous on the scalar ring.
                        prow = psr.tile([1, D], F32)
                        for jb in range(2):
                            for ki in range(KI):
                                if t == LAST:
                                    blk = wtiles[t][jb * 4 + ki // 2]
                                    rhs = blk[:, (ki % 2) * 512:
                                              (ki % 2) * 512 + 512]
                                else:
                                    wh = (wtiles[t][0] if ki < KH
                                          else wtiles[t][1])
                                    rhs = wh[:, (ki % KH) * D + jb * 512:
                                              (ki % KH) * D + jb * 512 + 512]
                                nc.tensor.matmul(
                                    prow[0:1, jb * 512:(jb + 1) * 512],
                                    lhsT=cur[:, ki:ki + 1],
                                    rhs=rhs,
                                    start=(ki == 0),
                                    stop=(ki == KI - 1),
                                )
                        orow = hrp.tile([1, D], F32, tag="orow")
                        for jb in range(2):
                            sl = slice(jb * 512, (jb + 1) * 512)
                            zrow = hrp.tile([1, 512], F32, tag="zrow")
                            nc.vector.tensor_add(
                                out=zrow,
                                in0=prow[0:1, sl],
                                in1=bmrt[0:1, b * 2 * D + jb * 512:
                                         b * 2 * D + (jb + 1) * 512],
                            )
                            trow = hrp.tile([1, 512], F32, tag="trow")
                            nc.vector.scalar_tensor_tensor(
                                out=trow,
                                in0=zrow,
                                scalar=0.0,
                                in1=bmrt[0:1, b * 2 * D + D + jb * 512:
                                         b * 2 * D + D + (jb + 1) * 512],
                                op0=mybir.AluOpType.min,
                                op1=mybir.AluOpType.mult,
                            )
                            nc.vector.tensor_sub(
                                out=orow[0:1, sl], in0=zrow, in1=trow)
                        nc.scalar.dma_start(
                            out=out[b:b + 1, :], in_=orow)
                        continue

                    # z = h @ W in 2 half ranges of 512 (one PSUM bank
                    # each, so the ranges never serialize against the
                    # other half's DVE drain).
                    prow = psr.tile([1, D], F32)
                    hrow = hrp.tile([1, D], F32)
                    for jb in range(2):
                        for ki in range(KI):
                            wh = wtiles[t][0] if ki < KH else wtiles[t][1]
                            base = (ki % KH) * D + jb * 512
                            nc.tensor.matmul(
                                prow[0:1, jb * 512:(jb + 1) * 512],
                                lhsT=cur[:, ki:ki + 1],
                                rhs=wh[:, base:base + 512],
                                start=(ki == 0),
                                stop=(ki == KI - 1),
                            )
                        # bias-fused PSUM->SBUF copy (row space)
                        nc.vector.tensor_add(
                            out=hrow[0:1, jb * 512:(jb + 1) * 512],
                            in0=prow[0:1, jb * 512:(jb + 1) * 512],
                            in1=brt[0:1, t * D + jb * 512:
                                     t * D + (jb + 1) * 512],
                        )

                    # row -> column transpose ON THE PE: 8 outer-product
                    # matmuls (lhsT = z-chunk [1,128] stationary, rhs =
                    # ones [1,1]) land z chunk m as PSUM column m.  No
                    # DMA in the layer chain, so the HWDGE completion
                    # lanes carry only the weight stream, and the chain
                    # latency is ~1.5us (well under the other sample's
                    # 3.4us matmul group).
                    pt = ptp.tile([P, KI], F32, tag="pt")
                    for m in range(KI):
                        nc.tensor.matmul(
                            pt[:, m:m + 1],
                            lhsT=hrow[0:1, m * P:(m + 1) * P],
                            rhs=onet[0:1, 0:1],
                            start=True,
                            stop=True,
                        )

                    # masked relu on the column tile: h = zb - mask*min(zb,0)
                    tmp = hcp.tile([P, KI], F32, tag="tmp")
                    hnew = hcp.tile([P, KI], hdt, tag="h")
                    nc.vector.scalar_tensor_tensor(
                        out=tmp,
                        in0=pt[:],
                        scalar=0.0,
                        in1=mkt[:, t * KI:(t + 1) * KI],
                        op0=mybir.AluOpType.min,
                        op1=mybir.AluOpType.mult,
                    )
                    nc.vector.tensor_sub(out=hnew, in0=pt[:], in1=tmp)
                    h[b] = hnew
    nc.finalize()
    return nc


def _get_nc():
    if WMODE not in _CACHE:
        _CACHE[WMODE] = _build(WMODE)
    return _CACHE[WMODE]


def _prep_core_inputs(c, x, weights, biases, masks):
    b0 = c * BC
    # weights[l, b, i, j], i = ki*128 + p  ->  [t, p, ki*1024 + j]
    # c-outer chunking (contraction chunk ki = consecutive 128 rows),
    # matching the PE-transpose column layout of h.
    wc = weights[:, b0:b0 + BC].reshape(L, BC, KI, P, D)
    wc = np.ascontiguousarray(wc.transpose(0, 1, 3, 2, 4)).reshape(
        NT, P, KI * D)
    # last tile jb-major: [p, ki*1024 + jb*512 + j'] -> [p, jb*4096 + ki*512 + j']
    wl = wc[NT - 1].reshape(P, KI, 2, 512).transpose(0, 2, 1, 3)
    wc[NT - 1] = np.ascontiguousarray(wl).reshape(P, KI * D)
    if WMODE == "bf16":
        import ml_dtypes
        wc = wc.astype(ml_dtypes.bfloat16)
    # x[b, ki*128+p] -> [p, b*KI + ki]  (c-outer)
    xc = x[b0:b0 + BC].reshape(BC, KI, P)
    xc = np.ascontiguousarray(xc.transpose(2, 0, 1)).reshape(P, BC * KI)
    if WMODE == "bf16":
        import ml_dtypes
        xc = xc.astype(ml_dtypes.bfloat16)
    # bias rows: [1, t*D + j]
    import ml_dtypes
    brc = np.ascontiguousarray(
        biases[:, b0:b0 + BC]).reshape(1, NT * D).astype(ml_dtypes.bfloat16)
    # masks column layout: [p, t*KI + ki], i = ki*128 + p  (c-outer)
    mc = masks[:, b0:b0 + BC].astype(np.float32).reshape(L, BC, KI, P)
    mc = np.ascontiguousarray(mc.transpose(3, 0, 1, 2)).reshape(P, NT * KI)
    # last layer's bias/mask, row-major per sample: [b, (bias | mask)]
    bmrc = np.concatenate(
        [biases[L - 1, b0:b0 + BC],
         masks[L - 1, b0:b0 + BC].astype(np.float32)],
        axis=1,
    )
    bmrc = np.ascontiguousarray(bmrc)
    onesc = np.ones((1, 1), dtype=np.float32)
    return {"w": wc, "x": xc, "br": brc, "mk": mc, "bmr": bmrc,
            "ones": onesc}


def _run(inputs: dict, trace: bool = False, trace_cores=None, tmpdir=None):
    x = np.asarray(inputs["x"], dtype=np.float32)
    weights = np.asarray(inputs["weights"], dtype=np.float32)
    biases = np.asarray(inputs["biases"], dtype=np.float32)
    masks = np.asarray(inputs["masks"])

    nc = _get_nc()
    in_maps = [
        _prep_core_inputs(c, x, weights, biases, masks) for c in range(NCORES)
    ]
    kw = {}
    if trace_cores is not None:
        kw["trace_cores"] = trace_cores
    if tmpdir is not None:
        kw["tmpdir"] = tmpdir
    res = run_bass_kernel_spmd(
        nc, in_maps, core_ids=list(range(NCORES)), trace=trace, **kw
    )
    outs = []
    for c in range(NCORES):
        oc = res.results[c]["out"]  # [BC, D] row-major
        outs.append(oc)
    full = np.concatenate(outs, axis=0).astype(np.float32)
    return full, res


def kernel(**inputs) -> np.ndarray:
    full, _ = _run(inputs, trace=False)
    return full
